# revision 1
# baseline (speedup 1.0000x reference)
"""Trainium2 Bass kernel for nn_BasicFlow (sparse window attention flow).

Sharding: pure data-parallel over batch B=8 -> one image pair per NeuronCore.
Device computes (per core, in bf16 on the PE):
  - 4x conv3x3 (128->128ch, 96x96) as 9 shifted accumulating matmuls
  - all 8 shift-variant x 144-window correlation matmuls (64x64 per window)
and writes the raw correlation volumes to DRAM. The small softmax/flow/splice/
bilinear tail (~1% of FLOPs) is vectorized numpy on host.
"""

import os

# recover wedged NeuronCores at NRT init (observed transient
# NRT_EXEC_UNIT_UNRECOVERABLE; reset-on-load clears it)
os.environ.setdefault("NEURON_RT_RESET_CORES", "1")

import numpy as np
import ml_dtypes

import concourse.bass as bass
import concourse.bacc as bacc
import concourse.tile as tile
import concourse.mybir as mybir
from concourse import bass_utils

F32 = mybir.dt.float32
BF16 = mybir.dt.bfloat16

B = 8
DIM = 128
H = W = 96
P = 8
UP = 4
SCALE = DIM ** -0.5
S1 = S2 = H // P          # 12 windows per axis
NW = S1 * S2              # 144 windows
NV = 8                    # 4 shift variants x 2 directions

_COMPILED = None


# --------------------------------------------------------------------------
# Device kernel
# --------------------------------------------------------------------------

def _build_device():
    nc = bacc.Bacc("TRN2", target_bir_lowering=False, debug=False, num_devices=8)

    f0_d = nc.dram_tensor("f0", [DIM, H, W], BF16, kind="ExternalInput")
    f2_d = nc.dram_tensor("f2", [DIM, H, W], BF16, kind="ExternalInput")
    wq_d = nc.dram_tensor("wq", [DIM, 9, DIM], BF16, kind="ExternalInput")
    wk_d = nc.dram_tensor("wk", [DIM, 9, DIM], BF16, kind="ExternalInput")
    bq_d = nc.dram_tensor("bq", [DIM, 1], F32, kind="ExternalInput")
    bk_d = nc.dram_tensor("bk", [DIM, 1], F32, kind="ExternalInput")
    # raw correlation volumes, window pairs packed across 128 partitions:
    # [variant*2+dir, par*64+q_pixel, window_pair, k_pixel], window = 2*pair+par
    corr_d = nc.dram_tensor("corr", [NV, 2 * P * P, NW // 2, P * P], BF16,
                            kind="ExternalOutput")

    with tile.TileContext(nc) as tc:
        with (
            tc.tile_pool(name="const", bufs=1) as constp,
            tc.tile_pool(name="big", bufs=5) as bigp,
            tc.tile_pool(name="qk", bufs=1) as qkp,
            tc.tile_pool(name="stage", bufs=10) as stagep,
            tc.tile_pool(name="psum", bufs=8, space="PSUM") as psump,
        ):
            wq_sb = constp.tile([DIM, 9, DIM], BF16, tag="wq")
            wk_sb = constp.tile([DIM, 9, DIM], BF16, tag="wk")
            bq_sb = constp.tile([DIM, 1], F32, tag="bq")
            bk_sb = constp.tile([DIM, 1], F32, tag="bk")
            nc.sync.dma_start(wq_sb[:], wq_d[:])
            nc.sync.dma_start(wk_sb[:], wk_d[:])
            nc.sync.dma_start(bq_sb[:], bq_d[:])
            nc.sync.dma_start(bk_sb[:], bk_d[:])

            q0 = qkp.tile([DIM, H, W], BF16, tag="q0")
            k0 = qkp.tile([DIM, H, W], BF16, tag="k0")
            q2 = qkp.tile([DIM, H, W], BF16, tag="q2")
            k2 = qkp.tile([DIM, H, W], BF16, tag="k2")

            RT = 4                       # output rows per psum tile
            NRT = H // RT

            def conv(dst, fpad, w_sb, b_sb):
                for rt in range(NRT):
                    ps = psump.tile([DIM, RT, W], F32, tag="ps")
                    for t in range(9):
                        dy, dx = divmod(t, 3)
                        rhs = fpad[:, rt * RT + dy: rt * RT + dy + RT,
                                   dx: dx + W]
                        nc.tensor.matmul(ps[:], w_sb[:, t, :], rhs,
                                         start=(t == 0), stop=(t == 8))
                    nc.scalar.activation(
                        dst[:, rt * RT:(rt + 1) * RT, :], ps[:],
                        mybir.ActivationFunctionType.Identity, bias=b_sb[:])

            # Load both padded feature maps upfront (slots shared with the
            # window-major tiles below), then conv in q0,k2,q2,k0 order so
            # the first correlation variant can start mid-conv-phase.
            fpads = []
            for src_d in (f0_d, f2_d):
                fpad = bigp.tile([DIM, H + 2, W + 2], BF16, tag="big")
                # zero only the 1-px border; interior is fully DMA-written
                nc.vector.memset(fpad[:, 0, :], 0.0)
                nc.vector.memset(fpad[:, H + 1, :], 0.0)
                nc.vector.memset(fpad[:, 1:H + 1, 0], 0.0)
                nc.vector.memset(fpad[:, 1:H + 1, W + 1], 0.0)
                # chunked load: first conv row-tiles start before the full
                # feature map lands
                CH = H // 4
                for c in range(4):
                    nc.sync.dma_start(
                        fpad[:, 1 + c * CH:1 + (c + 1) * CH, 1:W + 1],
                        src_d[:, c * CH:(c + 1) * CH, :])
                fpads.append(fpad)
            conv(q0, fpads[0], wq_sb, bq_sb)
            conv(k2, fpads[1], wk_sb, bk_sb)
            conv(q2, fpads[1], wq_sb, bq_sb)
            conv(k0, fpads[0], wk_sb, bk_sb)

            def _boxes(r):
                # (w0, nw, l0, nl) boxes over (window, local) of one axis so
                # that src rows w*8+l+r (mod 96) are contiguous per box
                if r == 0:
                    return [(0, S2, 0, P)]
                return [(0, S2 - 1, 0, P), (S2 - 1, 1, 0, P - r),
                        (S2 - 1, 1, P - r, r)]

            def wm_copy(dst, src, ry, rx, eng):
                # dst[ch, wy*12+wx, ly*8+lx] = src[ch, (wy*8+ly+ry)%96,
                #                                      (wx*8+lx+rx)%96]
                dstv = dst[:].rearrange("p (wy wx) (ly lx) -> p wy wx ly lx",
                                        wx=S2, lx=P)
                for wy0, nwy, ly0, nly in _boxes(ry):
                    for wx0, nwx, lx0, nlx in _boxes(rx):
                        d = dstv[:, wy0:wy0 + nwy, wx0:wx0 + nwx,
                                 ly0:ly0 + nly, lx0:lx0 + nlx]
                        r0 = (wy0 * P + ly0 + ry) % H
                        c0 = (wx0 * P + lx0 + rx) % W
                        s = src[:, r0:r0 + (nwy - 1) * P + nly,
                                c0:c0 + (nwx - 1) * P + nlx]
                        s = s.rearrange("p (wy ly) (wx lx) -> p wy wx ly lx",
                                        ly=nly, lx=nlx)
                        eng.tensor_copy(d, s)

            WG = 16                     # windows per psum bank (fills 2KB)
            for v in range(4):
                ry = 4 if v >= 2 else 0
                rx = 4 if (v % 2) else 0
                for d in range(2):
                    qs_base, ks_base = (q0, k2) if d == 0 else (q2, k0)
                    qs = bigp.tile([DIM, NW, P * P], BF16, tag="big")
                    ks = bigp.tile([DIM, NW, P * P], BF16, tag="big")
                    wm_copy(qs, qs_base, ry, rx, nc.vector)
                    wm_copy(ks, ks_base, ry, rx, nc.gpsimd)
                    vd = v * 2 + d
                    # even window -> PE col-groups 0-1 (psum partitions 0-63),
                    # odd window -> col-groups 2-3 (64-127); pairs run
                    # concurrently in the array
                    for wg in range(NW // WG):
                        ps = psump.tile([2 * P * P, WG // 2, P * P], F32,
                                        tag="ps")
                        sb = stagep.tile([2 * P * P, WG // 2, P * P], BF16,
                                         tag="corrsb")
                        for wi in range(WG // 2):
                            w = wg * WG + 2 * wi
                            nc.tensor.matmul(ps[0:64, wi, :], qs[:, w, :],
                                             ks[:, w, :], start=True,
                                             stop=True, tile_position=(0, 0))
                            nc.tensor.matmul(ps[64:128, wi, :],
                                             qs[:, w + 1, :], ks[:, w + 1, :],
                                             start=True, stop=True,
                                             tile_position=(0, 64))
                        if wg % 3 == 1:
                            nc.vector.tensor_copy(sb[:], ps[:])
                        else:
                            nc.scalar.copy(sb[:], ps[:])
                        nc.sync.dma_start(
                            corr_d[vd, :, wg * (WG // 2):(wg + 1) * (WG // 2),
                                   :], sb[:])

    nc.compile()
    return nc


def _run_device(feat0, feat2, wq, bq, wk, bk):
    global _COMPILED
    if _COMPILED is None:
        _COMPILED = _build_device()
    nc = _COMPILED

    bf = ml_dtypes.bfloat16
    wqT = np.ascontiguousarray(
        wq.astype(np.float32).transpose(1, 2, 3, 0).reshape(DIM, 9, DIM)
    ).astype(bf)
    wkT = np.ascontiguousarray(
        wk.astype(np.float32).transpose(1, 2, 3, 0).reshape(DIM, 9, DIM)
    ).astype(bf)
    bqc = np.ascontiguousarray(bq.astype(np.float32).reshape(DIM, 1))
    bkc = np.ascontiguousarray(bk.astype(np.float32).reshape(DIM, 1))

    in_maps = []
    for b in range(B):
        in_maps.append({
            "f0": np.ascontiguousarray(feat0[b]).astype(bf),
            "f2": np.ascontiguousarray(feat2[b]).astype(bf),
            "wq": wqT, "wk": wkT, "bq": bqc, "bk": bkc,
        })
    import os
    trace = bool(int(os.environ.get("BASSFLOW_TRACE", "0")))
    res = bass_utils.run_bass_kernel_spmd(nc, in_maps, core_ids=list(range(B)),
                                          trace=trace)
    if trace:
        print(f"HW exec time: {res.exec_time_ns} ns "
              f"(mean {res.mean_exec_time_ns})")
        if res.instructions_and_trace:
            print("trace path:", res.instructions_and_trace[1])
    corr = np.stack([res.results[b]["corr"] for b in range(B)])
    # [B, NV, par*64+q, pair, k] -> [B, NV, win=2*pair+par, q, k]
    corr = corr.reshape(B, NV, 2, P * P, NW // 2, P * P)
    corr = corr.transpose(0, 1, 4, 2, 3, 5).reshape(B, NV, NW, P * P, P * P)
    return corr.astype(np.float32)


# --------------------------------------------------------------------------
# Host tail: bias/mask + softmax flow pipeline + splice + bilinear upsample
# (numpy port of the reference; ~1% of total FLOPs)
# --------------------------------------------------------------------------

def _bias_index():
    coords = np.stack(np.meshgrid(np.arange(P), np.arange(P),
                                  indexing='ij')).reshape(2, -1)
    rel = (coords[:, :, None] - coords[:, None, :]).transpose(1, 2, 0).copy()
    rel[..., 0] += P - 1
    rel[..., 1] += P - 1
    rel[..., 0] *= 2 * P - 1
    return rel.sum(-1).reshape(-1)


def _pos():
    r = np.arange(P, dtype=np.float32)
    yy, xx = np.meshgrid(r, r, indexing='ij')
    return np.stack([xx, yy])[None].reshape(1, 2, P * P)


def _make_mask(Hp, Wp, sh, sw):
    m = np.zeros((Hp, Wp))
    hs = ((slice(0, -sh * 2), slice(-sh * 2, -sh), slice(-sh, None))
          if sh else (slice(None),))
    ws = ((slice(0, -sw * 2), slice(-sw * 2, -sw), slice(-sw, None))
          if sw else (slice(None),))
    cnt = 0
    for a in hs:
        for b in ws:
            m[a, b] = cnt
            cnt += 1
    win = m.reshape(Hp // P, P, Wp // P, P).transpose(0, 2, 1, 3).reshape(-1, P * P)
    d = win[:, None, :] - win[:, :, None]
    return np.where(d != 0, -10000.0, 0.0).astype(np.float32)


def _softmax(x, axis):
    m = np.max(x, axis=axis, keepdims=True)
    e = np.exp(x - m)
    return e / np.sum(e, axis=axis, keepdims=True)


_MID_IDX = None


def _mid_gather():
    """c[b, (j,k), (h2,w2)] = corr[b, (j+3-h2, k+3-w2), (h2,w2)] (0 if invalid)."""
    global _MID_IDX
    if _MID_IDX is None:
        j, k, h2, w2 = np.meshgrid(np.arange(9), np.arange(9), np.arange(P),
                                   np.arange(P), indexing='ij')
        qy = j + 3 - h2
        qx = k + 3 - w2
        valid = (qy >= 0) & (qy < P) & (qx >= 0) & (qx < P)
        qidx = np.clip(qy, 0, P - 1) * P + np.clip(qx, 0, P - 1)
        kidx = h2 * P + w2
        _MID_IDX = (qidx.reshape(81, 64), kidx.reshape(81, 64),
                    valid.reshape(81, 64))
    return _MID_IDX


def _flow_mid(corr, pos):
    bw = corr.shape[0]
    qidx, kidx, valid = _mid_gather()
    c = corr[:, qidx, kidx] * valid[None]          # (bw, 81, 64)
    n = P + 1
    r = np.arange(0.0, P - 0.5, 0.5)
    yy, xx = np.meshgrid(r, r, indexing='ij')
    CH = P // 2 - 1
    base = np.stack([xx, yy])[None][:, :, CH:2 * P - 1 - CH, CH:2 * P - 1 - CH]
    base = base.reshape(1, 2, n * n).astype(np.float32)
    flow = pos[:, :, None, :] - base[:, :, :, None]          # (1,2,81,64)
    smax = _softmax(c, axis=2)
    fl = np.einsum('bmk,cmk->bcm', smax, flow[0]).reshape(bw, 2, n, n)
    cr = np.sum(c * smax, axis=2).reshape(bw, 1, n, n)
    corr4 = np.concatenate([cr[:, :, :-1, :-1], cr[:, :, :-1, 1:],
                            cr[:, :, 1:, :-1], cr[:, :, 1:, 1:]], axis=1)
    flow4 = np.concatenate([fl[:, :, :-1, :-1], fl[:, :, :-1, 1:],
                            fl[:, :, 1:, :-1], fl[:, :, 1:, 1:]], axis=1)
    corr4 = corr4.transpose(0, 2, 3, 1).reshape(bw, P * P, 4)
    flow4 = flow4.reshape(bw, 4, 2, P, P).transpose(0, 2, 3, 4, 1)
    flow4 = flow4.reshape(bw, 2, P * P, 4) * 2
    smax2 = _softmax(corr4, axis=2)
    out = np.sum(flow4 * smax2[:, None], axis=3)
    return out.reshape(bw, 2, P, P).astype(np.float32)


def _flow_bsd(corr, pos):
    cut = P // 4
    bw = corr.shape[0]
    c = corr.reshape(bw, P, P, P * P)[:, cut:P - cut, cut:P - cut, :]
    L = (P - 2 * cut) ** 2
    c = c.reshape(bw, L, P * P)
    base = _pos().reshape(1, 2, P, P)[:, :, cut:P - cut, cut:P - cut]
    base = base.reshape(1, 2, L)
    flow = pos[:, :, None, :] - base[:, :, :, None]
    smax = _softmax(c, axis=2)
    out = np.einsum('blk,clk->bcl', smax, flow[0])
    return out.reshape(bw, 2, P - 2 * cut, P - 2 * cut).astype(np.float32)


def _splice(f00, f01, f10, f11, factor, Ho, Wo):
    f = np.concatenate([np.concatenate([f00, f01], axis=3),
                        np.concatenate([f10, f11], axis=3)], axis=2)
    bs, kk, hh, ww = f.shape
    b = bs // (S1 * S2)
    f = f.reshape(b, S1, S2, kk, hh, ww).transpose(0, 3, 1, 4, 2, 5)
    f = f.reshape(b, kk, S1 * hh, S2 * ww)
    sft = (P // 4) * factor
    f = np.roll(f, (sft, sft), axis=(2, 3))
    return f[:, :, :Ho * factor, :Wo * factor]


def _resize_mat(in_size, out_size):
    scale = out_size / in_size
    sample = (np.arange(out_size) + 0.5) / scale - 0.5
    x = np.abs(sample[None, :] - np.arange(in_size)[:, None])
    w = np.maximum(0.0, 1.0 - x)
    tot = w.sum(0, keepdims=True)
    return (w / np.where(tot == 0, 1.0, tot)).astype(np.float32)


def _up(x, f):
    b, c, h, w = x.shape
    My = _resize_mat(h, h * f)
    Mx = _resize_mat(w, w * f)
    y = np.einsum('bchw,hH->bcHw', x, My)
    y = np.einsum('bcHw,wW->bcHW', y, Mx)
    return (y * f).astype(np.float32)


def _host_flow(corr_raw, bias_table):
    """corr_raw: (B, NV, NW, 64, 64) raw q.k^T dot products."""
    bias = bias_table.astype(np.float32)[_bias_index()].reshape(
        P * P, P * P, 1).transpose(2, 0, 1)          # (1,64,64)
    pos = _pos()
    masks = {}
    for v, (sh, sw) in enumerate(((0, 0), (0, 4), (4, 0), (4, 4))):
        masks[v] = _make_mask(H, W, sh, sw) if (sh or sw) else None

    f1 = {}
    f0 = {}
    for v in range(4):
        for d in range(2):
            c = corr_raw[:, v * 2 + d].reshape(B * NW, 64, 64) * SCALE + bias
            if masks[v] is not None:
                c = (c.reshape(B, NW, 64, 64) + masks[v][None]).reshape(
                    B * NW, 64, 64)
            f1[(v, d)] = _flow_mid(c, pos)
            f0[(v, d)] = _flow_bsd(c, pos)

    # direction 0: (q0,k2) -> flow12 (mid), flow02 (bsd)
    # direction 1: (q2,k0) -> flow10 (mid), flow20 (bsd)
    flow12 = _splice(f1[(0, 0)], f1[(1, 0)], f1[(2, 0)], f1[(3, 0)], 2, H, W)
    flow02 = _splice(f0[(0, 0)], f0[(1, 0)], f0[(2, 0)], f0[(3, 0)], 1, H, W)
    flow10 = _splice(f1[(0, 1)], f1[(1, 1)], f1[(2, 1)], f1[(3, 1)], 2, H, W)
    flow20 = _splice(f0[(0, 1)], f0[(1, 1)], f0[(2, 1)], f0[(3, 1)], 1, H, W)
    fh, ff = UP // 2, UP
    return (_up(flow10, fh), _up(flow12, fh), _up(flow02, ff), _up(flow20, ff))


def kernel(feat0, feat2, wq, bq, wk, bk, bias_table):
    corr_raw = _run_device(np.asarray(feat0), np.asarray(feat2),
                           np.asarray(wq), np.asarray(bq),
                           np.asarray(wk), np.asarray(bk))
    return _host_flow(corr_raw, np.asarray(bias_table))



# revision 3
# speedup vs baseline: 1.8362x; 1.8362x over previous
"""Trainium2 Bass kernel for nn_BasicFlow (sparse window attention flow).

Sharding: pure data-parallel over batch B=8 -> one image pair per NeuronCore.
Device computes (per core):
  - 4x conv3x3 (128->128ch, 96x96) as row-strip matmuls over a host-prepadded
    input (bf16, or fp8 DoubleRow with optional error-feedback pass)
  - all 8 shift-variant x 144-window correlation matmuls in bf16; the k-side
    windows are read directly from the conv-output tile via strided APs
    (a 4px wrap halo removes the roll), the q-side is gathered window-major
    (one DVE copy per variant) because the PE stationary operand must be
    contiguous.
Raw correlation volumes go to DRAM; the small softmax/flow/splice/bilinear
tail (~1% of FLOPs) is vectorized numpy on host.
"""

import os

# recover wedged NeuronCores at NRT init (observed transient
# NRT_EXEC_UNIT_UNRECOVERABLE; reset-on-load clears it)
os.environ.setdefault("NEURON_RT_RESET_CORES", "1")

import numpy as np
import ml_dtypes
import bass_rust

import concourse.bass as bass
import concourse.bacc as bacc
import concourse.tile as tile
import concourse.mybir as mybir
from concourse import bass_utils

F32 = mybir.dt.float32
BF16 = mybir.dt.bfloat16
F8 = mybir.dt.float8e4
BFNP = ml_dtypes.bfloat16
F8NP = ml_dtypes.float8_e4m3fn

B = 8
DIM = 128
H = W = 96
P = 8
UP = 4
SCALE = DIM ** -0.5
S1 = S2 = H // P          # 12 windows per axis
NW = S1 * S2              # 144 windows
NV = 8                    # 4 shift variants x 2 directions
SHIFTS = ((0, 0), (0, 4), (4, 0), (4, 4))

XW = 98                   # padded input width (1px conv border)
XN = XW * XW + 4          # flat padded image + 4 elem slack for strip reads
QW = 100                  # conv output tile width (4px wrap halo)
RT = 4                    # conv output rows per psum tile
NRT = H // RT
NSTRIP = RT * XW          # 392: psum strip length per conv tile
DELTA = [dy * XW + dx for dy in range(3) for dx in range(3)]

# conv numerics: 'bf16' | 'fp8' (raw DoubleRow) | 'fp8ef' (input error feedback)
MODE = os.environ.get("BASSFLOW_MODE", "fp8ef")

_COMPILED = {}


def _strip_ap(xin, img, slot, nslots, y0, delta, pair_stride):
    """Moving AP for one conv row-strip: [128, (2,) NSTRIP] fp8/bf16 elements
    starting at flat offset 98*y0+delta of image `img`, slot `slot` (ef).
    pair_stride: None -> plain [128, N]; else DoubleRow [128, 2, N]."""
    base = xin[:]
    part = list(base.ap)[0]
    off = base.offset + (img * nslots + slot) * XN + XW * y0 + delta
    if pair_stride is None:
        dims = [list(part), [1, NSTRIP]]
    else:
        dims = [list(part), [pair_stride, 2], [1, NSTRIP]]
    ap = base.copy()
    ap.ap = bass_rust.VecI64Pair(dims)
    ap.offset = off
    return ap


# --------------------------------------------------------------------------
# Device kernel
# --------------------------------------------------------------------------

def _build_device(mode):
    nc = bacc.Bacc("TRN2", target_bir_lowering=False, debug=False, num_devices=8)

    nslots = 2 if mode == "fp8ef" else 1
    xdt = BF16 if mode == "bf16" else F8
    if mode == "bf16":
        wshape = [DIM, 9, DIM]
    elif mode == "fp8":
        wshape = [DIM, 5, 2, DIM]
    else:
        wshape = [DIM, 9, 2, DIM]

    xin_d = nc.dram_tensor("xin", [DIM, 2 * nslots * XN], xdt,
                           kind="ExternalInput")
    wq_d = nc.dram_tensor("wq", wshape, xdt, kind="ExternalInput")
    wk_d = nc.dram_tensor("wk", wshape, xdt, kind="ExternalInput")
    bq_d = nc.dram_tensor("bq", [DIM, 1], F32, kind="ExternalInput")
    bk_d = nc.dram_tensor("bk", [DIM, 1], F32, kind="ExternalInput")
    # raw correlation volumes, window pairs packed across 128 partitions:
    # [variant*2+dir, par*64+q_pixel, window_pair, k_pixel], window = 2*pair+par
    corr_d = nc.dram_tensor("corr", [NV, 2 * P * P, NW // 2, P * P], BF16,
                            kind="ExternalOutput")

    with tile.TileContext(nc) as tc:
        with (
            tc.tile_pool(name="const", bufs=1) as constp,
            tc.tile_pool(name="xin", bufs=1) as xinp,
            tc.tile_pool(name="qk", bufs=1) as qkp,
            tc.tile_pool(name="qwm", bufs=4) as qwmp,
            tc.tile_pool(name="stage", bufs=8) as stagep,
            tc.tile_pool(name="psum", bufs=8, space="PSUM") as psump,
        ):
            wq_sb = constp.tile(wshape, xdt, tag="wq")
            wk_sb = constp.tile(wshape, xdt, tag="wk")
            bq_sb = constp.tile([DIM, 1], F32, tag="bq")
            bk_sb = constp.tile([DIM, 1], F32, tag="bk")
            nc.sync.dma_start(wq_sb[:], wq_d[:])
            nc.sync.dma_start(wk_sb[:], wk_d[:])
            nc.sync.dma_start(bq_sb[:], bq_d[:])
            nc.sync.dma_start(bk_sb[:], bk_d[:])

            xin = xinp.tile([DIM, 2 * nslots * XN], xdt, tag="xin")
            # 2 chunks per (img, slot) so conv starts after the first half
            xv = xin[:].rearrange("p (s n) -> p s n", s=2 * nslots)
            xd = xin_d[:].rearrange("p (s n) -> p s n", s=2 * nslots)
            HC = XN // 2
            for s in range(2 * nslots):
                for c in range(2):
                    nc.sync.dma_start(xv[:, s, c * HC:(c + 1) * HC],
                                      xd[:, s, c * HC:(c + 1) * HC])

            # conv output tiles with 4px wrap halo (rows/cols 96:100 = 0:4)
            qe0 = qkp.tile([DIM, QW, QW], BF16, tag="qe0")
            ke2 = qkp.tile([DIM, QW, QW], BF16, tag="ke2")
            qe2 = qkp.tile([DIM, QW, QW], BF16, tag="qe2")
            ke0 = qkp.tile([DIM, QW, QW], BF16, tag="ke0")

            def conv(dst, img, w_sb, b_sb):
                for rt in range(NRT):
                    y0 = rt * RT
                    ps = psump.tile([DIM, NSTRIP], F32, tag="ps")
                    if mode == "bf16":
                        for t in range(9):
                            mov = _strip_ap(xin, img, 0, nslots, y0,
                                            DELTA[t], None)
                            nc.tensor.matmul(ps[:], w_sb[:, t, :], mov,
                                             start=(t == 0), stop=(t == 8))
                    elif mode == "fp8":
                        for p5 in range(5):
                            if p5 < 4:
                                a, b = 2 * p5, 2 * p5 + 1
                                stride = DELTA[b] - DELTA[a]
                            else:
                                a, stride = 8, 1  # partner is zero weights
                            mov = _strip_ap(xin, img, 0, nslots, y0,
                                            DELTA[a], stride)
                            nc.tensor.matmul(
                                ps[:], w_sb[:, p5, :, :], mov,
                                start=(p5 == 0), stop=(p5 == 4),
                                perf_mode=mybir.MatmulPerfMode.DoubleRow)
                    else:  # fp8ef: pair dim selects (X8, E8) image slot
                        for t in range(9):
                            mov = _strip_ap(xin, img, 0, nslots, y0,
                                            DELTA[t], XN)
                            nc.tensor.matmul(
                                ps[:], w_sb[:, t, :, :], mov,
                                start=(t == 0), stop=(t == 8),
                                perf_mode=mybir.MatmulPerfMode.DoubleRow)
                    psv = ps[:].rearrange("p (r c) -> p r c", r=RT)[:, :, 0:W]
                    nc.scalar.activation(
                        dst[:, y0:y0 + RT, 0:W], psv,
                        mybir.ActivationFunctionType.Identity, bias=b_sb[:])

            def halo(t):
                nc.vector.tensor_copy(t[:, W:QW, 0:W], t[:, 0:4, 0:W])
                nc.vector.tensor_copy(t[:, :, W:QW], t[:, :, 0:4])

            def gathers(qe):
                out = []
                for (ry, rx) in SHIFTS:
                    qwm = qwmp.tile([DIM, NW, P * P], BF16, tag="qwm")
                    src = qe[:, ry:ry + H, rx:rx + W].rearrange(
                        "p (wy ly) (wx lx) -> p wy wx ly lx", ly=P, lx=P)
                    dst = qwm[:].rearrange(
                        "p (wy wx) (ly lx) -> p wy wx ly lx", wx=S2, lx=P)
                    nc.vector.tensor_copy(dst, src)
                    out.append(qwm)
                return out

            conv(qe0, 0, wq_sb, bq_sb)
            halo(qe0)
            conv(ke2, 1, wk_sb, bk_sb)
            halo(ke2)
            qwm0 = gathers(qe0)
            conv(qe2, 1, wq_sb, bq_sb)
            halo(qe2)
            conv(ke0, 0, wk_sb, bk_sb)
            halo(ke0)
            qwm2 = gathers(qe2)

            WG = 16                     # windows per psum bank (fills 2KB)
            copy_engines = (nc.vector, nc.scalar, nc.gpsimd)
            ci = 0
            for d in range(2):
                qwm, ke = (qwm0, ke2) if d == 0 else (qwm2, ke0)
                for v, (ry, rx) in enumerate(SHIFTS):
                    vd = v * 2 + d
                    for wg in range(NW // WG):
                        ps = psump.tile([2 * P * P, WG // 2, P * P], F32,
                                        tag="ps")
                        sb = stagep.tile([2 * P * P, WG // 2, P * P], BF16,
                                         tag="corrsb")
                        for wi in range(WG // 2):
                            for par in range(2):
                                w = wg * WG + 2 * wi + par
                                wy, wx = divmod(w, S2)
                                mov = ke[:, ry + wy * P: ry + wy * P + P,
                                         rx + wx * P: rx + wx * P + P]
                                nc.tensor.matmul(
                                    ps[64 * par:64 * par + 64, wi, :],
                                    qwm[v][:, w, :], mov, start=True,
                                    stop=True, tile_position=(0, 64 * par))
                        eng = copy_engines[ci % 2]
                        ci += 1
                        if eng is nc.scalar:
                            eng.copy(sb[:], ps[:])
                        else:
                            eng.tensor_copy(sb[:], ps[:])
                        nc.sync.dma_start(
                            corr_d[vd, :, wg * (WG // 2):(wg + 1) * (WG // 2),
                                   :], sb[:])

    nc.compile()
    return nc


# --------------------------------------------------------------------------
# Host-side input prep + device run
# --------------------------------------------------------------------------

def _pad_flat(img):
    """img [DIM, H, W] f32 -> flat padded [DIM, XN] f32 (zero border+slack)."""
    xp = np.zeros((DIM, XN), np.float32)
    v = xp[:, :XW * XW].reshape(DIM, XW, XW)
    v[:, 1:H + 1, 1:W + 1] = img
    return xp


def _prep_weights(w, mode):
    """w (O,I,3,3) f32 -> device layout per mode."""
    wT = np.ascontiguousarray(
        w.astype(np.float32).transpose(1, 2, 3, 0).reshape(DIM, 9, DIM))
    if mode == "bf16":
        return wT.astype(BFNP)
    w8 = wT.astype(F8NP)
    if mode == "fp8ef":
        out = np.zeros((DIM, 9, 2, DIM), F8NP)
        out[:, :, 0, :] = w8
        out[:, :, 1, :] = w8
        return np.ascontiguousarray(out)
    out = np.zeros((DIM, 5, 2, DIM), F8NP)
    for p5 in range(4):
        out[:, p5, 0, :] = w8[:, 2 * p5, :]
        out[:, p5, 1, :] = w8[:, 2 * p5 + 1, :]
    out[:, 4, 0, :] = w8[:, 8, :]
    return np.ascontiguousarray(out)


def _prep_xin(f0, f2, mode):
    flats = [_pad_flat(f0), _pad_flat(f2)]
    if mode == "bf16":
        return np.ascontiguousarray(
            np.stack(flats, axis=1).reshape(DIM, 2 * XN)).astype(BFNP)
    if mode == "fp8":
        return np.ascontiguousarray(
            np.stack(flats, axis=1).reshape(DIM, 2 * XN)).astype(F8NP)
    slots = []
    for fl in flats:
        x8 = fl.astype(F8NP)
        e8 = (fl - x8.astype(np.float32)).astype(F8NP)
        slots += [x8, e8]
    return np.ascontiguousarray(
        np.stack(slots, axis=1).reshape(DIM, 4 * XN))


def _run_device(feat0, feat2, wq, bq, wk, bk):
    mode = MODE
    if mode not in _COMPILED:
        _COMPILED[mode] = _build_device(mode)
    nc = _COMPILED[mode]

    wqT = _prep_weights(wq, mode)
    wkT = _prep_weights(wk, mode)
    bqc = np.ascontiguousarray(bq.astype(np.float32).reshape(DIM, 1))
    bkc = np.ascontiguousarray(bk.astype(np.float32).reshape(DIM, 1))

    in_maps = []
    for b in range(B):
        in_maps.append({
            "xin": _prep_xin(np.asarray(feat0[b], np.float32),
                             np.asarray(feat2[b], np.float32), mode),
            "wq": wqT, "wk": wkT, "bq": bqc, "bk": bkc,
        })
    trace = bool(int(os.environ.get("BASSFLOW_TRACE", "0")))
    res = bass_utils.run_bass_kernel_spmd(nc, in_maps, core_ids=list(range(B)),
                                          trace=trace)
    if trace:
        print(f"HW exec time: {res.exec_time_ns} ns "
              f"(mean {res.mean_exec_time_ns})")
        if res.instructions_and_trace:
            print("trace path:", res.instructions_and_trace[1])
    corr = np.stack([res.results[b]["corr"] for b in range(B)])
    # [B, NV, par*64+q, pair, k] -> [B, NV, win=2*pair+par, q, k]
    corr = corr.reshape(B, NV, 2, P * P, NW // 2, P * P)
    corr = corr.transpose(0, 1, 4, 2, 3, 5).reshape(B, NV, NW, P * P, P * P)
    return corr.astype(np.float32)


# --------------------------------------------------------------------------
# Host tail: bias/mask + softmax flow pipeline + splice + bilinear upsample
# (numpy port of the reference; ~1% of total FLOPs)
# --------------------------------------------------------------------------

def _bias_index():
    coords = np.stack(np.meshgrid(np.arange(P), np.arange(P),
                                  indexing='ij')).reshape(2, -1)
    rel = (coords[:, :, None] - coords[:, None, :]).transpose(1, 2, 0).copy()
    rel[..., 0] += P - 1
    rel[..., 1] += P - 1
    rel[..., 0] *= 2 * P - 1
    return rel.sum(-1).reshape(-1)


def _pos():
    r = np.arange(P, dtype=np.float32)
    yy, xx = np.meshgrid(r, r, indexing='ij')
    return np.stack([xx, yy])[None].reshape(1, 2, P * P)


def _make_mask(Hp, Wp, sh, sw):
    m = np.zeros((Hp, Wp))
    hs = ((slice(0, -sh * 2), slice(-sh * 2, -sh), slice(-sh, None))
          if sh else (slice(None),))
    ws = ((slice(0, -sw * 2), slice(-sw * 2, -sw), slice(-sw, None))
          if sw else (slice(None),))
    cnt = 0
    for a in hs:
        for b in ws:
            m[a, b] = cnt
            cnt += 1
    win = m.reshape(Hp // P, P, Wp // P, P).transpose(0, 2, 1, 3).reshape(-1, P * P)
    d = win[:, None, :] - win[:, :, None]
    return np.where(d != 0, -10000.0, 0.0).astype(np.float32)


def _softmax(x, axis):
    m = np.max(x, axis=axis, keepdims=True)
    e = np.exp(x - m)
    return e / np.sum(e, axis=axis, keepdims=True)


_MID_IDX = None


def _mid_gather():
    """c[b, (j,k), (h2,w2)] = corr[b, (j+3-h2, k+3-w2), (h2,w2)] (0 if invalid)."""
    global _MID_IDX
    if _MID_IDX is None:
        j, k, h2, w2 = np.meshgrid(np.arange(9), np.arange(9), np.arange(P),
                                   np.arange(P), indexing='ij')
        qy = j + 3 - h2
        qx = k + 3 - w2
        valid = (qy >= 0) & (qy < P) & (qx >= 0) & (qx < P)
        qidx = np.clip(qy, 0, P - 1) * P + np.clip(qx, 0, P - 1)
        kidx = h2 * P + w2
        _MID_IDX = (qidx.reshape(81, 64), kidx.reshape(81, 64),
                    valid.reshape(81, 64))
    return _MID_IDX


def _flow_mid(corr, pos):
    bw = corr.shape[0]
    qidx, kidx, valid = _mid_gather()
    c = corr[:, qidx, kidx] * valid[None]          # (bw, 81, 64)
    n = P + 1
    r = np.arange(0.0, P - 0.5, 0.5)
    yy, xx = np.meshgrid(r, r, indexing='ij')
    CH = P // 2 - 1
    base = np.stack([xx, yy])[None][:, :, CH:2 * P - 1 - CH, CH:2 * P - 1 - CH]
    base = base.reshape(1, 2, n * n).astype(np.float32)
    flow = pos[:, :, None, :] - base[:, :, :, None]          # (1,2,81,64)
    smax = _softmax(c, axis=2)
    fl = np.einsum('bmk,cmk->bcm', smax, flow[0]).reshape(bw, 2, n, n)
    cr = np.sum(c * smax, axis=2).reshape(bw, 1, n, n)
    corr4 = np.concatenate([cr[:, :, :-1, :-1], cr[:, :, :-1, 1:],
                            cr[:, :, 1:, :-1], cr[:, :, 1:, 1:]], axis=1)
    flow4 = np.concatenate([fl[:, :, :-1, :-1], fl[:, :, :-1, 1:],
                            fl[:, :, 1:, :-1], fl[:, :, 1:, 1:]], axis=1)
    corr4 = corr4.transpose(0, 2, 3, 1).reshape(bw, P * P, 4)
    flow4 = flow4.reshape(bw, 4, 2, P, P).transpose(0, 2, 3, 4, 1)
    flow4 = flow4.reshape(bw, 2, P * P, 4) * 2
    smax2 = _softmax(corr4, axis=2)
    out = np.sum(flow4 * smax2[:, None], axis=3)
    return out.reshape(bw, 2, P, P).astype(np.float32)


def _flow_bsd(corr, pos):
    cut = P // 4
    bw = corr.shape[0]
    c = corr.reshape(bw, P, P, P * P)[:, cut:P - cut, cut:P - cut, :]
    L = (P - 2 * cut) ** 2
    c = c.reshape(bw, L, P * P)
    base = _pos().reshape(1, 2, P, P)[:, :, cut:P - cut, cut:P - cut]
    base = base.reshape(1, 2, L)
    flow = pos[:, :, None, :] - base[:, :, :, None]
    smax = _softmax(c, axis=2)
    out = np.einsum('blk,clk->bcl', smax, flow[0])
    return out.reshape(bw, 2, P - 2 * cut, P - 2 * cut).astype(np.float32)


def _splice(f00, f01, f10, f11, factor, Ho, Wo):
    f = np.concatenate([np.concatenate([f00, f01], axis=3),
                        np.concatenate([f10, f11], axis=3)], axis=2)
    bs, kk, hh, ww = f.shape
    b = bs // (S1 * S2)
    f = f.reshape(b, S1, S2, kk, hh, ww).transpose(0, 3, 1, 4, 2, 5)
    f = f.reshape(b, kk, S1 * hh, S2 * ww)
    sft = (P // 4) * factor
    f = np.roll(f, (sft, sft), axis=(2, 3))
    return f[:, :, :Ho * factor, :Wo * factor]


def _resize_mat(in_size, out_size):
    scale = out_size / in_size
    sample = (np.arange(out_size) + 0.5) / scale - 0.5
    x = np.abs(sample[None, :] - np.arange(in_size)[:, None])
    w = np.maximum(0.0, 1.0 - x)
    tot = w.sum(0, keepdims=True)
    return (w / np.where(tot == 0, 1.0, tot)).astype(np.float32)


def _up(x, f):
    b, c, h, w = x.shape
    My = _resize_mat(h, h * f)
    Mx = _resize_mat(w, w * f)
    y = np.einsum('bchw,hH->bcHw', x, My)
    y = np.einsum('bcHw,wW->bcHW', y, Mx)
    return (y * f).astype(np.float32)


def _host_flow(corr_raw, bias_table):
    """corr_raw: (B, NV, NW, 64, 64) raw q.k^T dot products."""
    bias = bias_table.astype(np.float32)[_bias_index()].reshape(
        P * P, P * P, 1).transpose(2, 0, 1)          # (1,64,64)
    pos = _pos()
    masks = {}
    for v, (sh, sw) in enumerate(((0, 0), (0, 4), (4, 0), (4, 4))):
        masks[v] = _make_mask(H, W, sh, sw) if (sh or sw) else None

    f1 = {}
    f0 = {}
    for v in range(4):
        for d in range(2):
            c = corr_raw[:, v * 2 + d].reshape(B * NW, 64, 64) * SCALE + bias
            if masks[v] is not None:
                c = (c.reshape(B, NW, 64, 64) + masks[v][None]).reshape(
                    B * NW, 64, 64)
            f1[(v, d)] = _flow_mid(c, pos)
            f0[(v, d)] = _flow_bsd(c, pos)

    # direction 0: (q0,k2) -> flow12 (mid), flow02 (bsd)
    # direction 1: (q2,k0) -> flow10 (mid), flow20 (bsd)
    flow12 = _splice(f1[(0, 0)], f1[(1, 0)], f1[(2, 0)], f1[(3, 0)], 2, H, W)
    flow02 = _splice(f0[(0, 0)], f0[(1, 0)], f0[(2, 0)], f0[(3, 0)], 1, H, W)
    flow10 = _splice(f1[(0, 1)], f1[(1, 1)], f1[(2, 1)], f1[(3, 1)], 2, H, W)
    flow20 = _splice(f0[(0, 1)], f0[(1, 1)], f0[(2, 1)], f0[(3, 1)], 1, H, W)
    fh, ff = UP // 2, UP
    return (_up(flow10, fh), _up(flow12, fh), _up(flow02, ff), _up(flow20, ff))


def kernel(feat0, feat2, wq, bq, wk, bk, bias_table):
    corr_raw = _run_device(np.asarray(feat0), np.asarray(feat2),
                           np.asarray(wq), np.asarray(bq),
                           np.asarray(wk), np.asarray(bk))
    return _host_flow(corr_raw, np.asarray(bias_table))


# revision 8
# speedup vs baseline: 1.8999x; 1.0347x over previous
"""Trainium2 Bass kernel for nn_BasicFlow (sparse window attention flow).

Sharding: pure data-parallel over batch B=8 -> one image pair per NeuronCore.
Device computes (per core):
  - 4x conv3x3 (128->128ch, 96x96) as row-strip matmuls over a host-prepadded
    input (bf16, or fp8 DoubleRow with optional error-feedback pass)
  - all 8 shift-variant x 144-window correlation matmuls in bf16; the k-side
    windows are read directly from the conv-output tile via strided APs
    (a 4px wrap halo removes the roll), the q-side is gathered window-major
    (one DVE copy per variant) because the PE stationary operand must be
    contiguous.
Raw correlation volumes go to DRAM; the small softmax/flow/splice/bilinear
tail (~1% of FLOPs) is vectorized numpy on host.
"""

import os

# recover wedged NeuronCores at NRT init (observed transient
# NRT_EXEC_UNIT_UNRECOVERABLE; reset-on-load clears it)
os.environ.setdefault("NEURON_RT_RESET_CORES", "1")

import numpy as np
import ml_dtypes
import bass_rust

import concourse.bass as bass
import concourse.bacc as bacc
import concourse.tile as tile
import concourse.mybir as mybir
from concourse import bass_utils

F32 = mybir.dt.float32
BF16 = mybir.dt.bfloat16
F8 = mybir.dt.float8e4
BFNP = ml_dtypes.bfloat16
F8NP = ml_dtypes.float8_e4m3fn

B = 8
DIM = 128
H = W = 96
P = 8
UP = 4
SCALE = DIM ** -0.5
S1 = S2 = H // P          # 12 windows per axis
NW = S1 * S2              # 144 windows
NV = 8                    # 4 shift variants x 2 directions
SHIFTS = ((0, 0), (0, 4), (4, 0), (4, 4))

XW = 98                   # padded input width (1px conv border)
XN = XW * XW + 4          # flat padded image + 4 elem slack for strip reads
QW = 100                  # conv output tile width (4px wrap halo)
RT = 4                    # conv output rows per psum tile
NRT = H // RT
NSTRIP = RT * XW          # 392: psum strip length per conv tile
DELTA = [dy * XW + dx for dy in range(3) for dx in range(3)]

# conv numerics: 'bf16' | 'fp8' (raw DoubleRow) | 'fp8ef' (input error feedback)
MODE = os.environ.get("BASSFLOW_MODE", "fp8ef")

_COMPILED = {}


def _strip_ap(xin, img, slot, nslots, y0, delta, pair_stride):
    """Moving AP for one conv row-strip: [128, (2,) NSTRIP] fp8/bf16 elements
    starting at flat offset 98*y0+delta of image `img`, slot `slot` (ef).
    pair_stride: None -> plain [128, N]; else DoubleRow [128, 2, N]."""
    base = xin[:]
    part = list(base.ap)[0]
    off = base.offset + (img * nslots + slot) * XN + XW * y0 + delta
    if pair_stride is None:
        dims = [list(part), [1, NSTRIP]]
    else:
        dims = [list(part), [pair_stride, 2], [1, NSTRIP]]
    ap = base.copy()
    ap.ap = bass_rust.VecI64Pair(dims)
    ap.offset = off
    return ap


# --------------------------------------------------------------------------
# Device kernel
# --------------------------------------------------------------------------

def _build_device(mode):
    nc = bacc.Bacc("TRN2", target_bir_lowering=False, debug=False, num_devices=8)

    nslots = 2 if mode == "fp8ef" else 1
    xdt = BF16 if mode == "bf16" else F8
    if mode == "bf16":
        wshape = [DIM, 9, DIM]
    elif mode == "fp8":
        wshape = [DIM, 5, 2, DIM]
    else:
        wshape = [DIM, 9, 2, DIM]

    xin_d = nc.dram_tensor("xin", [DIM, 2 * nslots * XN], xdt,
                           kind="ExternalInput")
    wq_d = nc.dram_tensor("wq", wshape, xdt, kind="ExternalInput")
    wk_d = nc.dram_tensor("wk", wshape, xdt, kind="ExternalInput")
    bq_d = nc.dram_tensor("bq", [DIM, 1], F32, kind="ExternalInput")
    bk_d = nc.dram_tensor("bk", [DIM, 1], F32, kind="ExternalInput")
    # raw correlation volumes, window pairs packed across 128 partitions:
    # [variant*2+dir, par*64+q_pixel, window_pair, k_pixel], window = 2*pair+par
    corr_d = nc.dram_tensor("corr", [NV, 2 * P * P, NW // 2, P * P], BF16,
                            kind="ExternalOutput")

    with tile.TileContext(nc) as tc:
        with (
            tc.tile_pool(name="const", bufs=1) as constp,
            tc.tile_pool(name="xin", bufs=1) as xinp,
            tc.tile_pool(name="qk", bufs=1) as qkp,
            tc.tile_pool(name="qwm", bufs=4) as qwmp,
            tc.tile_pool(name="stage", bufs=8) as stagep,
            tc.tile_pool(name="psum", bufs=8, space="PSUM") as psump,
        ):
            wq_sb = constp.tile(wshape, xdt, tag="wq")
            wk_sb = constp.tile(wshape, xdt, tag="wk")
            bq_sb = constp.tile([DIM, 1], F32, tag="bq")
            bk_sb = constp.tile([DIM, 1], F32, tag="bk")
            nc.sync.dma_start(wq_sb[:], wq_d[:])
            nc.sync.dma_start(bq_sb[:], bq_d[:])

            xin = xinp.tile([DIM, 2 * nslots * XN], xdt, tag="xin")
            # 4 chunks per (img, slot), chunk-major so conv q0 starts after
            # the first pair of quarter-chunks lands
            xv = xin[:].rearrange("p (s n) -> p s n", s=2 * nslots)
            xd = xin_d[:].rearrange("p (s n) -> p s n", s=2 * nslots)
            QC = XN // 4
            first = True
            for c in range(4):
                for s in range(2 * nslots):
                    hi = (c + 1) * QC if c < 3 else XN
                    nc.sync.dma_start(xv[:, s, c * QC:hi],
                                      xd[:, s, c * QC:hi])
                if first:
                    first = False
                    nc.sync.dma_start(wk_sb[:], wk_d[:])
                    nc.sync.dma_start(bk_sb[:], bk_d[:])

            # conv output tiles with 4px wrap halo (rows/cols 96:100 = 0:4)
            qe0 = qkp.tile([DIM, QW, QW], BF16, tag="qe0")
            ke2 = qkp.tile([DIM, QW, QW], BF16, tag="ke2")
            qe2 = qkp.tile([DIM, QW, QW], BF16, tag="qe2")
            ke0 = qkp.tile([DIM, QW, QW], BF16, tag="ke0")

            def conv(dst, img, w_sb, b_sb):
                for rt in range(NRT):
                    y0 = rt * RT
                    ps = psump.tile([DIM, NSTRIP], F32, tag="ps")
                    if mode == "bf16":
                        for t in range(9):
                            mov = _strip_ap(xin, img, 0, nslots, y0,
                                            DELTA[t], None)
                            nc.tensor.matmul(ps[:], w_sb[:, t, :], mov,
                                             start=(t == 0), stop=(t == 8))
                    elif mode == "fp8":
                        for p5 in range(5):
                            if p5 < 4:
                                a, b = 2 * p5, 2 * p5 + 1
                                stride = DELTA[b] - DELTA[a]
                            else:
                                a, stride = 8, 1  # partner is zero weights
                            mov = _strip_ap(xin, img, 0, nslots, y0,
                                            DELTA[a], stride)
                            nc.tensor.matmul(
                                ps[:], w_sb[:, p5, :, :], mov,
                                start=(p5 == 0), stop=(p5 == 4),
                                perf_mode=mybir.MatmulPerfMode.DoubleRow)
                    else:  # fp8ef: pair dim selects (X8, E8) image slot
                        for t in range(9):
                            mov = _strip_ap(xin, img, 0, nslots, y0,
                                            DELTA[t], XN)
                            nc.tensor.matmul(
                                ps[:], w_sb[:, t, :, :], mov,
                                start=(t == 0), stop=(t == 8),
                                perf_mode=mybir.MatmulPerfMode.DoubleRow)
                    psv = ps[:].rearrange("p (r c) -> p r c", r=RT)[:, :, 0:W]
                    nc.scalar.activation(
                        dst[:, y0:y0 + RT, 0:W], psv,
                        mybir.ActivationFunctionType.Identity, bias=b_sb[:])

            def halo(t):
                # on gpsimd: keeps the DVE queue free for the qwm gathers
                nc.gpsimd.tensor_copy(t[:, W:QW, 0:W], t[:, 0:4, 0:W])
                nc.gpsimd.tensor_copy(t[:, :, W:QW], t[:, :, 0:4])

            def gathers(qe):
                out = []
                for (ry, rx) in SHIFTS:
                    qwm = qwmp.tile([DIM, NW, P * P], BF16, tag="qwm")
                    src = qe[:, ry:ry + H, rx:rx + W].rearrange(
                        "p (wy ly) (wx lx) -> p wy wx ly lx", ly=P, lx=P)
                    dst = qwm[:].rearrange(
                        "p (wy wx) (ly lx) -> p wy wx ly lx", wx=S2, lx=P)
                    nc.vector.tensor_copy(dst, src)
                    out.append(qwm)
                return out

            conv(qe0, 0, wq_sb, bq_sb)
            halo(qe0)
            conv(ke2, 1, wk_sb, bk_sb)
            halo(ke2)
            qwm0 = gathers(qe0)
            conv(qe2, 1, wq_sb, bq_sb)
            halo(qe2)
            qwm2 = gathers(qe2)
            conv(ke0, 0, wk_sb, bk_sb)
            halo(ke0)

            WG = 16                     # windows per psum bank (fills 2KB)
            copy_engines = (nc.vector, nc.scalar, nc.gpsimd)
            ci = 0
            for d in range(2):
                qwm, ke = (qwm0, ke2) if d == 0 else (qwm2, ke0)
                for v, (ry, rx) in enumerate(SHIFTS):
                    vd = v * 2 + d
                    for wg in range(NW // WG):
                        ps = psump.tile([2 * P * P, WG // 2, P * P], F32,
                                        tag="ps")
                        sb = stagep.tile([2 * P * P, WG // 2, P * P], BF16,
                                         tag="corrsb")
                        for wi in range(WG // 2):
                            for par in range(2):
                                w = wg * WG + 2 * wi + par
                                wy, wx = divmod(w, S2)
                                mov = ke[:, ry + wy * P: ry + wy * P + P,
                                         rx + wx * P: rx + wx * P + P]
                                nc.tensor.matmul(
                                    ps[64 * par:64 * par + 64, wi, :],
                                    qwm[v][:, w, :], mov, start=True,
                                    stop=True, tile_position=(0, 64 * par))
                        eng = copy_engines[ci % 2]
                        ci += 1
                        if eng is nc.scalar:
                            eng.copy(sb[:], ps[:])
                        else:
                            eng.tensor_copy(sb[:], ps[:])
                        nc.sync.dma_start(
                            corr_d[vd, :, wg * (WG // 2):(wg + 1) * (WG // 2),
                                   :], sb[:])

    nc.compile()
    return nc


# --------------------------------------------------------------------------
# Host-side input prep + device run
# --------------------------------------------------------------------------

def _pad_flat(img):
    """img [DIM, H, W] f32 -> flat padded [DIM, XN] f32 (zero border+slack)."""
    xp = np.zeros((DIM, XN), np.float32)
    v = xp[:, :XW * XW].reshape(DIM, XW, XW)
    v[:, 1:H + 1, 1:W + 1] = img
    return xp


def _prep_weights(w, mode):
    """w (O,I,3,3) f32 -> device layout per mode."""
    wT = np.ascontiguousarray(
        w.astype(np.float32).transpose(1, 2, 3, 0).reshape(DIM, 9, DIM))
    if mode == "bf16":
        return wT.astype(BFNP)
    w8 = wT.astype(F8NP)
    if mode == "fp8ef":
        out = np.zeros((DIM, 9, 2, DIM), F8NP)
        out[:, :, 0, :] = w8
        out[:, :, 1, :] = w8
        return np.ascontiguousarray(out)
    out = np.zeros((DIM, 5, 2, DIM), F8NP)
    for p5 in range(4):
        out[:, p5, 0, :] = w8[:, 2 * p5, :]
        out[:, p5, 1, :] = w8[:, 2 * p5 + 1, :]
    out[:, 4, 0, :] = w8[:, 8, :]
    return np.ascontiguousarray(out)


def _prep_xin(f0, f2, mode):
    flats = [_pad_flat(f0), _pad_flat(f2)]
    if mode == "bf16":
        return np.ascontiguousarray(
            np.stack(flats, axis=1).reshape(DIM, 2 * XN)).astype(BFNP)
    if mode == "fp8":
        return np.ascontiguousarray(
            np.stack(flats, axis=1).reshape(DIM, 2 * XN)).astype(F8NP)
    slots = []
    for fl in flats:
        x8 = fl.astype(F8NP)
        e8 = (fl - x8.astype(np.float32)).astype(F8NP)
        slots += [x8, e8]
    return np.ascontiguousarray(
        np.stack(slots, axis=1).reshape(DIM, 4 * XN))


def _run_device(feat0, feat2, wq, bq, wk, bk):
    mode = MODE
    if mode not in _COMPILED:
        _COMPILED[mode] = _build_device(mode)
    nc = _COMPILED[mode]

    wqT = _prep_weights(wq, mode)
    wkT = _prep_weights(wk, mode)
    bqc = np.ascontiguousarray(bq.astype(np.float32).reshape(DIM, 1))
    bkc = np.ascontiguousarray(bk.astype(np.float32).reshape(DIM, 1))

    in_maps = []
    for b in range(B):
        in_maps.append({
            "xin": _prep_xin(np.asarray(feat0[b], np.float32),
                             np.asarray(feat2[b], np.float32), mode),
            "wq": wqT, "wk": wkT, "bq": bqc, "bk": bkc,
        })
    trace = bool(int(os.environ.get("BASSFLOW_TRACE", "0")))
    res = bass_utils.run_bass_kernel_spmd(nc, in_maps, core_ids=list(range(B)),
                                          trace=trace)
    if trace:
        print(f"HW exec time: {res.exec_time_ns} ns "
              f"(mean {res.mean_exec_time_ns})")
        if res.instructions_and_trace:
            print("trace path:", res.instructions_and_trace[1])
    corr = np.stack([res.results[b]["corr"] for b in range(B)])
    # [B, NV, par*64+q, pair, k] -> [B, NV, win=2*pair+par, q, k]
    corr = corr.reshape(B, NV, 2, P * P, NW // 2, P * P)
    corr = corr.transpose(0, 1, 4, 2, 3, 5).reshape(B, NV, NW, P * P, P * P)
    return corr.astype(np.float32)


# --------------------------------------------------------------------------
# Host tail: bias/mask + softmax flow pipeline + splice + bilinear upsample
# (numpy port of the reference; ~1% of total FLOPs)
# --------------------------------------------------------------------------

def _bias_index():
    coords = np.stack(np.meshgrid(np.arange(P), np.arange(P),
                                  indexing='ij')).reshape(2, -1)
    rel = (coords[:, :, None] - coords[:, None, :]).transpose(1, 2, 0).copy()
    rel[..., 0] += P - 1
    rel[..., 1] += P - 1
    rel[..., 0] *= 2 * P - 1
    return rel.sum(-1).reshape(-1)


def _pos():
    r = np.arange(P, dtype=np.float32)
    yy, xx = np.meshgrid(r, r, indexing='ij')
    return np.stack([xx, yy])[None].reshape(1, 2, P * P)


def _make_mask(Hp, Wp, sh, sw):
    m = np.zeros((Hp, Wp))
    hs = ((slice(0, -sh * 2), slice(-sh * 2, -sh), slice(-sh, None))
          if sh else (slice(None),))
    ws = ((slice(0, -sw * 2), slice(-sw * 2, -sw), slice(-sw, None))
          if sw else (slice(None),))
    cnt = 0
    for a in hs:
        for b in ws:
            m[a, b] = cnt
            cnt += 1
    win = m.reshape(Hp // P, P, Wp // P, P).transpose(0, 2, 1, 3).reshape(-1, P * P)
    d = win[:, None, :] - win[:, :, None]
    return np.where(d != 0, -10000.0, 0.0).astype(np.float32)


def _softmax(x, axis):
    m = np.max(x, axis=axis, keepdims=True)
    e = np.exp(x - m)
    return e / np.sum(e, axis=axis, keepdims=True)


_MID_IDX = None


def _mid_gather():
    """c[b, (j,k), (h2,w2)] = corr[b, (j+3-h2, k+3-w2), (h2,w2)] (0 if invalid)."""
    global _MID_IDX
    if _MID_IDX is None:
        j, k, h2, w2 = np.meshgrid(np.arange(9), np.arange(9), np.arange(P),
                                   np.arange(P), indexing='ij')
        qy = j + 3 - h2
        qx = k + 3 - w2
        valid = (qy >= 0) & (qy < P) & (qx >= 0) & (qx < P)
        qidx = np.clip(qy, 0, P - 1) * P + np.clip(qx, 0, P - 1)
        kidx = h2 * P + w2
        _MID_IDX = (qidx.reshape(81, 64), kidx.reshape(81, 64),
                    valid.reshape(81, 64))
    return _MID_IDX


def _flow_mid(corr, pos):
    bw = corr.shape[0]
    qidx, kidx, valid = _mid_gather()
    c = corr[:, qidx, kidx] * valid[None]          # (bw, 81, 64)
    n = P + 1
    r = np.arange(0.0, P - 0.5, 0.5)
    yy, xx = np.meshgrid(r, r, indexing='ij')
    CH = P // 2 - 1
    base = np.stack([xx, yy])[None][:, :, CH:2 * P - 1 - CH, CH:2 * P - 1 - CH]
    base = base.reshape(1, 2, n * n).astype(np.float32)
    flow = pos[:, :, None, :] - base[:, :, :, None]          # (1,2,81,64)
    smax = _softmax(c, axis=2)
    fl = np.einsum('bmk,cmk->bcm', smax, flow[0]).reshape(bw, 2, n, n)
    cr = np.sum(c * smax, axis=2).reshape(bw, 1, n, n)
    corr4 = np.concatenate([cr[:, :, :-1, :-1], cr[:, :, :-1, 1:],
                            cr[:, :, 1:, :-1], cr[:, :, 1:, 1:]], axis=1)
    flow4 = np.concatenate([fl[:, :, :-1, :-1], fl[:, :, :-1, 1:],
                            fl[:, :, 1:, :-1], fl[:, :, 1:, 1:]], axis=1)
    corr4 = corr4.transpose(0, 2, 3, 1).reshape(bw, P * P, 4)
    flow4 = flow4.reshape(bw, 4, 2, P, P).transpose(0, 2, 3, 4, 1)
    flow4 = flow4.reshape(bw, 2, P * P, 4) * 2
    smax2 = _softmax(corr4, axis=2)
    out = np.sum(flow4 * smax2[:, None], axis=3)
    return out.reshape(bw, 2, P, P).astype(np.float32)


def _flow_bsd(corr, pos):
    cut = P // 4
    bw = corr.shape[0]
    c = corr.reshape(bw, P, P, P * P)[:, cut:P - cut, cut:P - cut, :]
    L = (P - 2 * cut) ** 2
    c = c.reshape(bw, L, P * P)
    base = _pos().reshape(1, 2, P, P)[:, :, cut:P - cut, cut:P - cut]
    base = base.reshape(1, 2, L)
    flow = pos[:, :, None, :] - base[:, :, :, None]
    smax = _softmax(c, axis=2)
    out = np.einsum('blk,clk->bcl', smax, flow[0])
    return out.reshape(bw, 2, P - 2 * cut, P - 2 * cut).astype(np.float32)


def _splice(f00, f01, f10, f11, factor, Ho, Wo):
    f = np.concatenate([np.concatenate([f00, f01], axis=3),
                        np.concatenate([f10, f11], axis=3)], axis=2)
    bs, kk, hh, ww = f.shape
    b = bs // (S1 * S2)
    f = f.reshape(b, S1, S2, kk, hh, ww).transpose(0, 3, 1, 4, 2, 5)
    f = f.reshape(b, kk, S1 * hh, S2 * ww)
    sft = (P // 4) * factor
    f = np.roll(f, (sft, sft), axis=(2, 3))
    return f[:, :, :Ho * factor, :Wo * factor]


def _resize_mat(in_size, out_size):
    scale = out_size / in_size
    sample = (np.arange(out_size) + 0.5) / scale - 0.5
    x = np.abs(sample[None, :] - np.arange(in_size)[:, None])
    w = np.maximum(0.0, 1.0 - x)
    tot = w.sum(0, keepdims=True)
    return (w / np.where(tot == 0, 1.0, tot)).astype(np.float32)


def _up(x, f):
    b, c, h, w = x.shape
    My = _resize_mat(h, h * f)
    Mx = _resize_mat(w, w * f)
    y = np.einsum('bchw,hH->bcHw', x, My)
    y = np.einsum('bcHw,wW->bcHW', y, Mx)
    return (y * f).astype(np.float32)


def _host_flow(corr_raw, bias_table):
    """corr_raw: (B, NV, NW, 64, 64) raw q.k^T dot products."""
    bias = bias_table.astype(np.float32)[_bias_index()].reshape(
        P * P, P * P, 1).transpose(2, 0, 1)          # (1,64,64)
    pos = _pos()
    masks = {}
    for v, (sh, sw) in enumerate(((0, 0), (0, 4), (4, 0), (4, 4))):
        masks[v] = _make_mask(H, W, sh, sw) if (sh or sw) else None

    f1 = {}
    f0 = {}
    for v in range(4):
        for d in range(2):
            c = corr_raw[:, v * 2 + d].reshape(B * NW, 64, 64) * SCALE + bias
            if masks[v] is not None:
                c = (c.reshape(B, NW, 64, 64) + masks[v][None]).reshape(
                    B * NW, 64, 64)
            f1[(v, d)] = _flow_mid(c, pos)
            f0[(v, d)] = _flow_bsd(c, pos)

    # direction 0: (q0,k2) -> flow12 (mid), flow02 (bsd)
    # direction 1: (q2,k0) -> flow10 (mid), flow20 (bsd)
    flow12 = _splice(f1[(0, 0)], f1[(1, 0)], f1[(2, 0)], f1[(3, 0)], 2, H, W)
    flow02 = _splice(f0[(0, 0)], f0[(1, 0)], f0[(2, 0)], f0[(3, 0)], 1, H, W)
    flow10 = _splice(f1[(0, 1)], f1[(1, 1)], f1[(2, 1)], f1[(3, 1)], 2, H, W)
    flow20 = _splice(f0[(0, 1)], f0[(1, 1)], f0[(2, 1)], f0[(3, 1)], 1, H, W)
    fh, ff = UP // 2, UP
    return (_up(flow10, fh), _up(flow12, fh), _up(flow02, ff), _up(flow20, ff))


def kernel(feat0, feat2, wq, bq, wk, bk, bias_table):
    corr_raw = _run_device(np.asarray(feat0), np.asarray(feat2),
                           np.asarray(wq), np.asarray(bq),
                           np.asarray(wk), np.asarray(bk))
    return _host_flow(corr_raw, np.asarray(bias_table))


# revision 12
# speedup vs baseline: 2.0605x; 1.0846x over previous
"""Trainium2 Bass kernel for nn_BasicFlow (sparse window attention flow).

Sharding: pure data-parallel over batch B=8 -> one image pair per NeuronCore.
Device computes (per core):
  - 4x conv3x3 (128->128ch, 96x96) as row-strip matmuls over a host-prepadded
    input (bf16, or fp8 DoubleRow with optional error-feedback pass)
  - all 8 shift-variant x 144-window correlation matmuls in bf16; the k-side
    windows are read directly from the conv-output tile via strided APs
    (a 4px wrap halo removes the roll), the q-side is gathered window-major
    (one DVE copy per variant) because the PE stationary operand must be
    contiguous.
Raw correlation volumes go to DRAM; the small softmax/flow/splice/bilinear
tail (~1% of FLOPs) is vectorized numpy on host.
"""

import os

# recover wedged NeuronCores at NRT init (observed transient
# NRT_EXEC_UNIT_UNRECOVERABLE; reset-on-load clears it)
os.environ.setdefault("NEURON_RT_RESET_CORES", "1")

import numpy as np
import ml_dtypes
import bass_rust

import concourse.bass as bass
import concourse.bacc as bacc
import concourse.tile as tile
import concourse.mybir as mybir
from concourse import bass_utils

F32 = mybir.dt.float32
BF16 = mybir.dt.bfloat16
F8 = mybir.dt.float8e4
BFNP = ml_dtypes.bfloat16
F8NP = ml_dtypes.float8_e4m3fn

B = 8
DIM = 128
H = W = 96
P = 8
UP = 4
SCALE = DIM ** -0.5
S1 = S2 = H // P          # 12 windows per axis
NW = S1 * S2              # 144 windows
NV = 8                    # 4 shift variants x 2 directions
SHIFTS = ((0, 0), (0, 4), (4, 0), (4, 4))

XW = 98                   # padded input width (1px conv border)
XN = XW * XW + 4          # flat padded image + 4 elem slack for strip reads
QW = 100                  # conv output tile width (4px wrap halo)
RT = 4                    # conv output rows per psum tile
NRT = H // RT
NSTRIP = RT * XW          # 392: psum strip length per conv tile
DELTA = [dy * XW + dx for dy in range(3) for dx in range(3)]

# conv numerics: 'bf16' | 'fp8' (raw DoubleRow) | 'fp8ef' (input error feedback)
MODE = os.environ.get("BASSFLOW_MODE", "fp8ef")

_COMPILED = {}


def _strip_ap(xin, img, slot, nslots, y0, delta, pair_stride):
    """Moving AP for one conv row-strip: [128, (2,) NSTRIP] fp8/bf16 elements
    starting at flat offset 98*y0+delta of image `img`, slot `slot` (ef).
    pair_stride: None -> plain [128, N]; else DoubleRow [128, 2, N]."""
    base = xin[:]
    part = list(base.ap)[0]
    off = base.offset + (img * nslots + slot) * XN + XW * y0 + delta
    if pair_stride is None:
        dims = [list(part), [1, NSTRIP]]
    else:
        dims = [list(part), [pair_stride, 2], [1, NSTRIP]]
    ap = base.copy()
    ap.ap = bass_rust.VecI64Pair(dims)
    ap.offset = off
    return ap


# --------------------------------------------------------------------------
# Device kernel
# --------------------------------------------------------------------------

def _build_device(mode):
    nc = bacc.Bacc("TRN2", target_bir_lowering=False, debug=False, num_devices=8)

    nslots = 2 if mode == "fp8ef" else 1
    xdt = BF16 if mode == "bf16" else F8
    if mode == "bf16":
        wshape = [DIM, 9, DIM]
    elif mode == "fp8":
        wshape = [DIM, 5, 2, DIM]
    else:
        wshape = [DIM, 9, 2, DIM]

    xin_d = nc.dram_tensor("xin", [DIM, 2 * nslots * XN], xdt,
                           kind="ExternalInput")
    wq_d = nc.dram_tensor("wq", wshape, xdt, kind="ExternalInput")
    wk_d = nc.dram_tensor("wk", wshape, xdt, kind="ExternalInput")
    bq_d = nc.dram_tensor("bq", [DIM, 1], F32, kind="ExternalInput")
    bk_d = nc.dram_tensor("bk", [DIM, 1], F32, kind="ExternalInput")
    # raw correlation volumes, window pairs packed across 128 partitions:
    # [variant*2+dir, par*64+q_pixel, window_pair, k_pixel], window = 2*pair+par
    corr_d = nc.dram_tensor("corr", [NV, 2 * P * P, NW // 2, P * P], BF16,
                            kind="ExternalOutput")

    with tile.TileContext(nc) as tc:
        with (
            tc.tile_pool(name="const", bufs=1) as constp,
            tc.tile_pool(name="xin", bufs=1) as xinp,
            tc.tile_pool(name="qk", bufs=1) as qkp,
            tc.tile_pool(name="qwm", bufs=4) as qwmp,
            tc.tile_pool(name="stage", bufs=4) as stagep,
            tc.tile_pool(name="psum", bufs=8, space="PSUM") as psump,
        ):
            wq_sb = constp.tile(wshape, xdt, tag="wq")
            wk_sb = constp.tile(wshape, xdt, tag="wk")
            bq_sb = constp.tile([DIM, 1], F32, tag="bq")
            bk_sb = constp.tile([DIM, 1], F32, tag="bk")
            nc.sync.dma_start(wq_sb[:], wq_d[:])

            xin = xinp.tile([DIM, 2 * nslots * XN], xdt, tag="xin")
            # 8 chunks per (img, slot), chunk-major so conv q0 starts after
            # the first pair of eighth-chunks lands
            xv = xin[:].rearrange("p (s n) -> p s n", s=2 * nslots)
            xd = xin_d[:].rearrange("p (s n) -> p s n", s=2 * nslots)
            NCH = 8
            QC = XN // NCH
            for c in range(NCH):
                for s in range(2 * nslots):
                    hi = (c + 1) * QC if c < NCH - 1 else XN
                    nc.sync.dma_start(xv[:, s, c * QC:hi],
                                      xd[:, s, c * QC:hi])
                if c == 0:
                    nc.sync.dma_start(bq_sb[:], bq_d[:])
                elif c == 1:
                    nc.sync.dma_start(wk_sb[:], wk_d[:])
                    nc.sync.dma_start(bk_sb[:], bk_d[:])

            # conv output tiles with 4px wrap halo (rows/cols 96:100 = 0:4)
            qe0 = qkp.tile([DIM, QW, QW], BF16, tag="qe0")
            ke2 = qkp.tile([DIM, QW, QW], BF16, tag="ke2")
            qe2 = qkp.tile([DIM, QW, QW], BF16, tag="qe2")
            ke0 = qkp.tile([DIM, QW, QW], BF16, tag="ke0")

            def conv(dst, img, w_sb, b_sb):
                for rt in range(NRT):
                    y0 = rt * RT
                    ps = psump.tile([DIM, NSTRIP], F32, tag="ps")
                    if mode == "bf16":
                        for t in range(9):
                            mov = _strip_ap(xin, img, 0, nslots, y0,
                                            DELTA[t], None)
                            nc.tensor.matmul(ps[:], w_sb[:, t, :], mov,
                                             start=(t == 0), stop=(t == 8))
                    elif mode == "fp8":
                        for p5 in range(5):
                            if p5 < 4:
                                a, b = 2 * p5, 2 * p5 + 1
                                stride = DELTA[b] - DELTA[a]
                            else:
                                a, stride = 8, 1  # partner is zero weights
                            mov = _strip_ap(xin, img, 0, nslots, y0,
                                            DELTA[a], stride)
                            nc.tensor.matmul(
                                ps[:], w_sb[:, p5, :, :], mov,
                                start=(p5 == 0), stop=(p5 == 4),
                                perf_mode=mybir.MatmulPerfMode.DoubleRow)
                    else:  # fp8ef: pair dim selects (X8, E8) image slot
                        for t in range(9):
                            mov = _strip_ap(xin, img, 0, nslots, y0,
                                            DELTA[t], XN)
                            nc.tensor.matmul(
                                ps[:], w_sb[:, t, :, :], mov,
                                start=(t == 0), stop=(t == 8),
                                perf_mode=mybir.MatmulPerfMode.DoubleRow)
                    psv = ps[:].rearrange("p (r c) -> p r c", r=RT)[:, :, 0:W]
                    nc.scalar.activation(
                        dst[:, y0:y0 + RT, 0:W], psv,
                        mybir.ActivationFunctionType.Identity, bias=b_sb[:])

            def halo(t):
                # on gpsimd: keeps the DVE queue free for the qwm gathers
                nc.gpsimd.tensor_copy(t[:, W:QW, 0:W], t[:, 0:4, 0:W])
                nc.gpsimd.tensor_copy(t[:, :, W:QW], t[:, :, 0:4])

            def gathers(qe, engines=(nc.vector,) * 4):
                out = []
                for (ry, rx), eng in zip(SHIFTS, engines):
                    qwm = qwmp.tile([DIM, NW, P * P], BF16, tag="qwm")
                    src = qe[:, ry:ry + H, rx:rx + W].rearrange(
                        "p (wy ly) (wx lx) -> p wy wx ly lx", ly=P, lx=P)
                    dst = qwm[:].rearrange(
                        "p (wy wx) (ly lx) -> p wy wx ly lx", wx=S2, lx=P)
                    eng.tensor_copy(dst, src)
                    out.append(qwm)
                return out

            conv(qe0, 0, wq_sb, bq_sb)
            halo(qe0)
            conv(ke2, 1, wk_sb, bk_sb)
            halo(ke2)
            qwm0 = gathers(qe0)
            conv(qe2, 1, wq_sb, bq_sb)
            halo(qe2)
            # 2 gathers on gpsimd so the DVE is free for corr psum copies
            qwm2 = gathers(qe2, (nc.vector, nc.gpsimd, nc.vector, nc.gpsimd))
            conv(ke0, 0, wk_sb, bk_sb)
            halo(ke0)

            WG = 16                     # windows per psum bank (fills 2KB)
            NG = NW // WG               # 9 psum groups per variant
            DB = 3                      # psum groups batched per output DMA
            copy_engines = (nc.vector, nc.scalar)
            ci = 0
            for d in range(2):
                qwm, ke = (qwm0, ke2) if d == 0 else (qwm2, ke0)
                for v, (ry, rx) in enumerate(SHIFTS):
                    vd = v * 2 + d
                    for wg in range(NG):
                        ps = psump.tile([2 * P * P, WG // 2, P * P], F32,
                                        tag="ps")
                        if wg % DB == 0:
                            sb = stagep.tile(
                                [2 * P * P, DB * WG // 2, P * P], BF16,
                                tag="corrsb")
                        for wi in range(WG // 2):
                            for par in range(2):
                                w = wg * WG + 2 * wi + par
                                wy, wx = divmod(w, S2)
                                mov = ke[:, ry + wy * P: ry + wy * P + P,
                                         rx + wx * P: rx + wx * P + P]
                                nc.tensor.matmul(
                                    ps[64 * par:64 * par + 64, wi, :],
                                    qwm[v][:, w, :], mov, start=True,
                                    stop=True, tile_position=(0, 64 * par))
                        eng = copy_engines[ci % 2]
                        ci += 1
                        g = wg % DB
                        dst = sb[:, g * (WG // 2):(g + 1) * (WG // 2), :]
                        if eng is nc.scalar:
                            eng.copy(dst, ps[:])
                        else:
                            eng.tensor_copy(dst, ps[:])
                        if g == DB - 1:
                            w0 = (wg - DB + 1) * (WG // 2)
                            nc.sync.dma_start(
                                corr_d[vd, :, w0:w0 + DB * (WG // 2), :],
                                sb[:])

    nc.compile()
    return nc


# --------------------------------------------------------------------------
# Host-side input prep + device run
# --------------------------------------------------------------------------

def _pad_flat(img):
    """img [DIM, H, W] f32 -> flat padded [DIM, XN] f32 (zero border+slack)."""
    xp = np.zeros((DIM, XN), np.float32)
    v = xp[:, :XW * XW].reshape(DIM, XW, XW)
    v[:, 1:H + 1, 1:W + 1] = img
    return xp


def _prep_weights(w, mode):
    """w (O,I,3,3) f32 -> device layout per mode."""
    wT = np.ascontiguousarray(
        w.astype(np.float32).transpose(1, 2, 3, 0).reshape(DIM, 9, DIM))
    if mode == "bf16":
        return wT.astype(BFNP)
    w8 = wT.astype(F8NP)
    if mode == "fp8ef":
        out = np.zeros((DIM, 9, 2, DIM), F8NP)
        out[:, :, 0, :] = w8
        out[:, :, 1, :] = w8
        return np.ascontiguousarray(out)
    out = np.zeros((DIM, 5, 2, DIM), F8NP)
    for p5 in range(4):
        out[:, p5, 0, :] = w8[:, 2 * p5, :]
        out[:, p5, 1, :] = w8[:, 2 * p5 + 1, :]
    out[:, 4, 0, :] = w8[:, 8, :]
    return np.ascontiguousarray(out)


def _prep_xin(f0, f2, mode):
    flats = [_pad_flat(f0), _pad_flat(f2)]
    if mode == "bf16":
        return np.ascontiguousarray(
            np.stack(flats, axis=1).reshape(DIM, 2 * XN)).astype(BFNP)
    if mode == "fp8":
        return np.ascontiguousarray(
            np.stack(flats, axis=1).reshape(DIM, 2 * XN)).astype(F8NP)
    slots = []
    for fl in flats:
        x8 = fl.astype(F8NP)
        e8 = (fl - x8.astype(np.float32)).astype(F8NP)
        slots += [x8, e8]
    return np.ascontiguousarray(
        np.stack(slots, axis=1).reshape(DIM, 4 * XN))


def _run_device(feat0, feat2, wq, bq, wk, bk):
    mode = MODE
    if mode not in _COMPILED:
        _COMPILED[mode] = _build_device(mode)
    nc = _COMPILED[mode]

    wqT = _prep_weights(wq, mode)
    wkT = _prep_weights(wk, mode)
    bqc = np.ascontiguousarray(bq.astype(np.float32).reshape(DIM, 1))
    bkc = np.ascontiguousarray(bk.astype(np.float32).reshape(DIM, 1))

    in_maps = []
    for b in range(B):
        in_maps.append({
            "xin": _prep_xin(np.asarray(feat0[b], np.float32),
                             np.asarray(feat2[b], np.float32), mode),
            "wq": wqT, "wk": wkT, "bq": bqc, "bk": bkc,
        })
    trace = bool(int(os.environ.get("BASSFLOW_TRACE", "0")))
    res = bass_utils.run_bass_kernel_spmd(nc, in_maps, core_ids=list(range(B)),
                                          trace=trace)
    if trace:
        print(f"HW exec time: {res.exec_time_ns} ns "
              f"(mean {res.mean_exec_time_ns})")
        if res.instructions_and_trace:
            print("trace path:", res.instructions_and_trace[1])
    corr = np.stack([res.results[b]["corr"] for b in range(B)])
    # [B, NV, par*64+q, pair, k] -> [B, NV, win=2*pair+par, q, k]
    corr = corr.reshape(B, NV, 2, P * P, NW // 2, P * P)
    corr = corr.transpose(0, 1, 4, 2, 3, 5).reshape(B, NV, NW, P * P, P * P)
    return corr.astype(np.float32)


# --------------------------------------------------------------------------
# Host tail: bias/mask + softmax flow pipeline + splice + bilinear upsample
# (numpy port of the reference; ~1% of total FLOPs)
# --------------------------------------------------------------------------

def _bias_index():
    coords = np.stack(np.meshgrid(np.arange(P), np.arange(P),
                                  indexing='ij')).reshape(2, -1)
    rel = (coords[:, :, None] - coords[:, None, :]).transpose(1, 2, 0).copy()
    rel[..., 0] += P - 1
    rel[..., 1] += P - 1
    rel[..., 0] *= 2 * P - 1
    return rel.sum(-1).reshape(-1)


def _pos():
    r = np.arange(P, dtype=np.float32)
    yy, xx = np.meshgrid(r, r, indexing='ij')
    return np.stack([xx, yy])[None].reshape(1, 2, P * P)


def _make_mask(Hp, Wp, sh, sw):
    m = np.zeros((Hp, Wp))
    hs = ((slice(0, -sh * 2), slice(-sh * 2, -sh), slice(-sh, None))
          if sh else (slice(None),))
    ws = ((slice(0, -sw * 2), slice(-sw * 2, -sw), slice(-sw, None))
          if sw else (slice(None),))
    cnt = 0
    for a in hs:
        for b in ws:
            m[a, b] = cnt
            cnt += 1
    win = m.reshape(Hp // P, P, Wp // P, P).transpose(0, 2, 1, 3).reshape(-1, P * P)
    d = win[:, None, :] - win[:, :, None]
    return np.where(d != 0, -10000.0, 0.0).astype(np.float32)


def _softmax(x, axis):
    m = np.max(x, axis=axis, keepdims=True)
    e = np.exp(x - m)
    return e / np.sum(e, axis=axis, keepdims=True)


_MID_IDX = None


def _mid_gather():
    """c[b, (j,k), (h2,w2)] = corr[b, (j+3-h2, k+3-w2), (h2,w2)] (0 if invalid)."""
    global _MID_IDX
    if _MID_IDX is None:
        j, k, h2, w2 = np.meshgrid(np.arange(9), np.arange(9), np.arange(P),
                                   np.arange(P), indexing='ij')
        qy = j + 3 - h2
        qx = k + 3 - w2
        valid = (qy >= 0) & (qy < P) & (qx >= 0) & (qx < P)
        qidx = np.clip(qy, 0, P - 1) * P + np.clip(qx, 0, P - 1)
        kidx = h2 * P + w2
        _MID_IDX = (qidx.reshape(81, 64), kidx.reshape(81, 64),
                    valid.reshape(81, 64))
    return _MID_IDX


def _flow_mid(corr, pos):
    bw = corr.shape[0]
    qidx, kidx, valid = _mid_gather()
    c = corr[:, qidx, kidx] * valid[None]          # (bw, 81, 64)
    n = P + 1
    r = np.arange(0.0, P - 0.5, 0.5)
    yy, xx = np.meshgrid(r, r, indexing='ij')
    CH = P // 2 - 1
    base = np.stack([xx, yy])[None][:, :, CH:2 * P - 1 - CH, CH:2 * P - 1 - CH]
    base = base.reshape(1, 2, n * n).astype(np.float32)
    flow = pos[:, :, None, :] - base[:, :, :, None]          # (1,2,81,64)
    smax = _softmax(c, axis=2)
    fl = np.einsum('bmk,cmk->bcm', smax, flow[0]).reshape(bw, 2, n, n)
    cr = np.sum(c * smax, axis=2).reshape(bw, 1, n, n)
    corr4 = np.concatenate([cr[:, :, :-1, :-1], cr[:, :, :-1, 1:],
                            cr[:, :, 1:, :-1], cr[:, :, 1:, 1:]], axis=1)
    flow4 = np.concatenate([fl[:, :, :-1, :-1], fl[:, :, :-1, 1:],
                            fl[:, :, 1:, :-1], fl[:, :, 1:, 1:]], axis=1)
    corr4 = corr4.transpose(0, 2, 3, 1).reshape(bw, P * P, 4)
    flow4 = flow4.reshape(bw, 4, 2, P, P).transpose(0, 2, 3, 4, 1)
    flow4 = flow4.reshape(bw, 2, P * P, 4) * 2
    smax2 = _softmax(corr4, axis=2)
    out = np.sum(flow4 * smax2[:, None], axis=3)
    return out.reshape(bw, 2, P, P).astype(np.float32)


def _flow_bsd(corr, pos):
    cut = P // 4
    bw = corr.shape[0]
    c = corr.reshape(bw, P, P, P * P)[:, cut:P - cut, cut:P - cut, :]
    L = (P - 2 * cut) ** 2
    c = c.reshape(bw, L, P * P)
    base = _pos().reshape(1, 2, P, P)[:, :, cut:P - cut, cut:P - cut]
    base = base.reshape(1, 2, L)
    flow = pos[:, :, None, :] - base[:, :, :, None]
    smax = _softmax(c, axis=2)
    out = np.einsum('blk,clk->bcl', smax, flow[0])
    return out.reshape(bw, 2, P - 2 * cut, P - 2 * cut).astype(np.float32)


def _splice(f00, f01, f10, f11, factor, Ho, Wo):
    f = np.concatenate([np.concatenate([f00, f01], axis=3),
                        np.concatenate([f10, f11], axis=3)], axis=2)
    bs, kk, hh, ww = f.shape
    b = bs // (S1 * S2)
    f = f.reshape(b, S1, S2, kk, hh, ww).transpose(0, 3, 1, 4, 2, 5)
    f = f.reshape(b, kk, S1 * hh, S2 * ww)
    sft = (P // 4) * factor
    f = np.roll(f, (sft, sft), axis=(2, 3))
    return f[:, :, :Ho * factor, :Wo * factor]


def _resize_mat(in_size, out_size):
    scale = out_size / in_size
    sample = (np.arange(out_size) + 0.5) / scale - 0.5
    x = np.abs(sample[None, :] - np.arange(in_size)[:, None])
    w = np.maximum(0.0, 1.0 - x)
    tot = w.sum(0, keepdims=True)
    return (w / np.where(tot == 0, 1.0, tot)).astype(np.float32)


def _up(x, f):
    b, c, h, w = x.shape
    My = _resize_mat(h, h * f)
    Mx = _resize_mat(w, w * f)
    y = np.einsum('bchw,hH->bcHw', x, My)
    y = np.einsum('bcHw,wW->bcHW', y, Mx)
    return (y * f).astype(np.float32)


def _host_flow(corr_raw, bias_table):
    """corr_raw: (B, NV, NW, 64, 64) raw q.k^T dot products."""
    bias = bias_table.astype(np.float32)[_bias_index()].reshape(
        P * P, P * P, 1).transpose(2, 0, 1)          # (1,64,64)
    pos = _pos()
    masks = {}
    for v, (sh, sw) in enumerate(((0, 0), (0, 4), (4, 0), (4, 4))):
        masks[v] = _make_mask(H, W, sh, sw) if (sh or sw) else None

    f1 = {}
    f0 = {}
    for v in range(4):
        for d in range(2):
            c = corr_raw[:, v * 2 + d].reshape(B * NW, 64, 64) * SCALE + bias
            if masks[v] is not None:
                c = (c.reshape(B, NW, 64, 64) + masks[v][None]).reshape(
                    B * NW, 64, 64)
            f1[(v, d)] = _flow_mid(c, pos)
            f0[(v, d)] = _flow_bsd(c, pos)

    # direction 0: (q0,k2) -> flow12 (mid), flow02 (bsd)
    # direction 1: (q2,k0) -> flow10 (mid), flow20 (bsd)
    flow12 = _splice(f1[(0, 0)], f1[(1, 0)], f1[(2, 0)], f1[(3, 0)], 2, H, W)
    flow02 = _splice(f0[(0, 0)], f0[(1, 0)], f0[(2, 0)], f0[(3, 0)], 1, H, W)
    flow10 = _splice(f1[(0, 1)], f1[(1, 1)], f1[(2, 1)], f1[(3, 1)], 2, H, W)
    flow20 = _splice(f0[(0, 1)], f0[(1, 1)], f0[(2, 1)], f0[(3, 1)], 1, H, W)
    fh, ff = UP // 2, UP
    return (_up(flow10, fh), _up(flow12, fh), _up(flow02, ff), _up(flow20, ff))


def kernel(feat0, feat2, wq, bq, wk, bk, bias_table):
    corr_raw = _run_device(np.asarray(feat0), np.asarray(feat2),
                           np.asarray(wq), np.asarray(bq),
                           np.asarray(wk), np.asarray(bk))
    return _host_flow(corr_raw, np.asarray(bias_table))
